# revision 19
# baseline (speedup 1.0000x reference)
"""Trainium2 Bass kernel for nn_Dipole (multi-hot embedding + BiGRU + attention + FC).

Self-contained: hardcodes shapes B=32, T=100, V=10000, D=128, OUT=1000, 8 cores.
Sharding: data-parallel over batch (4 patients per core); weights replicated.

Per-core pipeline (all layouts d-major [128, ...], time-major free columns):
  1. e.T accumulated in fp32 PSUM from bf16 operands (batchdata is exactly
     representable in bf16; emb is split hi+lo bf16 so the pair sums to the
     fp32 value within ~2^-17 relative).
  2. x3 = e @ wih.T + (all foldable biases), computed fp32 and LEFT RESIDENT
     in PSUM banks; backward direction stored time-reversed so both
     directions share column blocks during the scan.
  3. GRU scan: per tick, 6 small fp32 matmuls accumulate whh @ h directly
     onto the x3 PSUM columns (has_written accumulate), one fused sigmoid
     over r/z x both dirs, one tanh, 5 DVE ops - all [128, 2*4] tiles.
  4. Attention scores via matmul, segmented softmax in [4,100] layout (DRAM
     bounce for partition reshape), masking + last-index selection, context
     via K=1 broadcast matmuls + tensor_tensor_reduce, then comb/fc matmuls.
"""

import sys

sys.path.insert(0, "/opt/trn_rl_repo")

import numpy as np
import ml_dtypes

import concourse.bass as bass
import concourse.bacc as bacc
import concourse.tile as tile
from concourse import mybir
from concourse import bass_utils
from concourse.bass_interp import get_hw_module

F32 = mybir.dt.float32
BF16 = mybir.dt.bfloat16
AF = mybir.ActivationFunctionType
ALU = mybir.AluOpType
AX = mybir.AxisListType

B, T, V, D, OUT = 32, 100, 10000, 128, 1000
NCORES = 8
BPC = B // NCORES          # 4 patients per core
N = BPC * T                # 400 free columns (time-major: col = t*BPC + b)
KT = (V + 127) // 128      # 79 k-tiles
VP = KT * 128              # 10112 padded vocab
KB = 8                     # k-tiles per DMA batch
NKB = (KT + KB - 1) // KB

bf16 = ml_dtypes.bfloat16

_STAGES = {"e": 1, "x3": 2, "scan": 3, "scores": 4, "soft": 5, "ab": 6, "ctx": 6, "feat": 7, "full": 9}


def build_nc(upto="full"):
    """Emit + compile the Bass program (single NEFF, SPMD across 8 cores).
    upto < full truncates after that stage and dumps a debug view."""
    lvl = _STAGES[upto]
    nc = bacc.Bacc("TRN2", target_bir_lowering=False, debug=False,
                   enable_asserts=False)

    # ---- DRAM I/O ----
    d_xt = nc.dram_tensor("xt", [KT, 128, N], BF16, kind="ExternalInput").ap()
    d_emb = nc.dram_tensor("embhl", [KT, 128, 256], BF16, kind="ExternalInput").ap()
    d_wih_f = nc.dram_tensor("wihT_f", [128, 384], F32, kind="ExternalInput").ap()
    d_wih_b = nc.dram_tensor("wihT_b", [128, 384], F32, kind="ExternalInput").ap()
    d_whh_f = nc.dram_tensor("whhT_f", [128, 384], F32, kind="ExternalInput").ap()
    d_whh_b = nc.dram_tensor("whhT_b", [128, 384], F32, kind="ExternalInput").ap()
    d_brow = nc.dram_tensor("bias_rows", [1, 768], F32, kind="ExternalInput").ap()
    d_bihn = nc.dram_tensor("bihn", [128, 2], F32, kind="ExternalInput").ap()
    d_attnw = nc.dram_tensor("attn_wc", [128, 2], F32, kind="ExternalInput").ap()
    d_attnb = nc.dram_tensor("attn_b4", [4, 1], F32, kind="ExternalInput").ap()
    d_combw = nc.dram_tensor("comb_wT", [128, 512], F32, kind="ExternalInput").ap()
    d_combb = nc.dram_tensor("comb_b", [128, 1], F32, kind="ExternalInput").ap()
    d_fcw = nc.dram_tensor("fc_wT", [128, OUT], F32, kind="ExternalInput").ap()
    d_fcb = nc.dram_tensor("fc_b", [1, OUT], F32, kind="ExternalInput").ap()
    d_iota = nc.dram_tensor("iota4", [4, T], F32, kind="ExternalInput").ap()
    d_out = nc.dram_tensor("logits", [BPC, OUT], F32, kind="ExternalOutput").ap()
    d_b1 = nc.dram_tensor("bounce1", [2 * N], F32, kind="Internal").ap()
    d_b2 = nc.dram_tensor("bounce2", [2 * N], F32, kind="Internal").ap()

    from contextlib import ExitStack
    with tile.TileContext(nc) as tc, ExitStack() as ctx:
        cm_x3 = tc.tile_pool(name="p_x3", bufs=1, space="PSUM")
        p_x3 = cm_x3.__enter__()
        cm_e = tc.tile_pool(name="p_e", bufs=2, space="PSUM")
        p_e = cm_e.__enter__()
        sb_c = ctx.enter_context(tc.tile_pool(name="sb_c", bufs=1))
        sb_m = ctx.enter_context(tc.tile_pool(name="sb_m", bufs=1))
        sb_scan = ctx.enter_context(tc.tile_pool(name="sb_scan", bufs=2))
        sb_x = ctx.enter_context(tc.tile_pool(name="sb_x", bufs=3))
        sb_e = ctx.enter_context(tc.tile_pool(name="sb_e", bufs=3))

        # ---- constants into SBUF ----
        w_sb = sb_c.tile([128, 4, 384], F32)       # wihf, wihb, whhf, whhb
        nc.sync.dma_start(out=w_sb[:, 0, :], in_=d_wih_f)
        nc.sync.dma_start(out=w_sb[:, 1, :], in_=d_wih_b)
        nc.sync.dma_start(out=w_sb[:, 2, :], in_=d_whh_f)
        nc.sync.dma_start(out=w_sb[:, 3, :], in_=d_whh_b)
        brow_sb = sb_c.tile([1, 768], F32)
        nc.sync.dma_start(out=brow_sb, in_=d_brow)
        bihn_sb = sb_c.tile([128, 2], F32)
        nc.sync.dma_start(out=bihn_sb, in_=d_bihn)
        attnw_sb = sb_c.tile([128, 2], F32)
        nc.sync.dma_start(out=attnw_sb, in_=d_attnw)
        attnb_sb = sb_c.tile([4, 1], F32)
        nc.sync.dma_start(out=attnb_sb, in_=d_attnb)
        combw_sb = sb_c.tile([128, 512], F32)
        nc.sync.dma_start(out=combw_sb, in_=d_combw)
        combb_sb = sb_c.tile([128, 1], F32)
        nc.sync.dma_start(out=combb_sb, in_=d_combb)
        fcw_sb = sb_c.tile([128, OUT], F32)
        nc.sync.dma_start(out=fcw_sb, in_=d_fcw)
        fcb_sb = sb_c.tile([1, OUT], F32)
        nc.sync.dma_start(out=fcb_sb, in_=d_fcb)
        iota_sb = sb_c.tile([4, T], F32)
        nc.sync.dma_start(out=iota_sb, in_=d_iota)
        ones_sb = sb_c.tile([1, N], F32)
        nc.vector.memset(ones_sb, 1.0)
        onescol_sb = sb_c.tile([128, 1], F32)
        nc.vector.memset(onescol_sb, 1.0)

        # ---- long-lived SBUF state ----
        e_sb = sb_m.tile([128, N], F32)            # e.T, col = t*BPC + b
        xn_sb = sb_m.tile([128, 2, N], F32)        # xn + bih_n; dir b time-reversed
        HS = sb_m.tile([128, T + 1, 2, BPC], F32)  # [.,tau,0,.]=hf(tau-1), [.,tau,1,.]=hb(T-tau)
        HSb = sb_m.tile([128, T, BPC], F32)        # hb in true time order

        def dump(src_ap, nfree):
            dbg = sb_m.tile([BPC, OUT], F32)
            nc.vector.memset(dbg, 0.0)
            nc.vector.tensor_copy(dbg[:, 0:nfree], src_ap)
            nc.sync.dma_start(out=d_out, in_=dbg)

        # ---- phase 1: e.T accumulation in PSUM ----
        e_ps = p_e.tile([128, N], F32, tag="escratch")
        for kb in range(NKB):
            nk = min(KB, KT - kb * KB)
            xk = sb_x.tile([128, KB, N], BF16)
            nc.sync.dma_start(
                out=xk[:, :nk, :],
                in_=d_xt[kb * KB:kb * KB + nk].rearrange("k p n -> p k n"))
            ek = sb_e.tile([128, KB, 256], BF16)
            nc.sync.dma_start(
                out=ek[:, :nk, :],
                in_=d_emb[kb * KB:kb * KB + nk].rearrange("k p n -> p k n"))
            for j in range(nk):
                k = kb * KB + j
                nc.tensor.matmul(e_ps, ek[:, j, 0:128], xk[:, j, :],
                                 start=(k == 0), stop=False)
                nc.tensor.matmul(e_ps, ek[:, j, 128:256], xk[:, j, :],
                                 start=False, stop=(k == KT - 1))
        nc.scalar.copy(e_sb, e_ps)
        if lvl == 1:
            dump(e_sb[0:BPC, :], N)

        if lvl >= 2:
            # reversed-time copy of e.T (hardware rejects negative-stride
            # APs; materialize the reversal with per-block copies)
            e_rev = sb_m.tile([128, N], F32)
            for tt in range(T):
                src_blk = e_sb[:, (T - 1 - tt) * BPC:(T - tt) * BPC]
                dst_blk = e_rev[:, tt * BPC:(tt + 1) * BPC]
                if tt % 2 == 0:
                    nc.vector.tensor_copy(dst_blk, src_blk)
                else:
                    nc.gpsimd.tensor_copy(dst_blk, src_blk)

            # ---- phase 2: x3 resident in PSUM + xn in SBUF ----
            rz_ps = p_x3.tile([128, 2, 2, 512], F32)   # [dir][gate r,z]
            n_ps = p_x3.tile([128, 2, 512], F32)       # [dir]
            for di in range(2):
                rhs_e = e_sb if di == 0 else e_rev
                for g in range(2):  # r, z
                    nc.tensor.matmul(rz_ps[:, di, g, 0:N],
                                     w_sb[:, di, g * 128:(g + 1) * 128], rhs_e,
                                     start=True, stop=True)
                    idx = di * 2 + g
                    nc.tensor.matmul(rz_ps[:, di, g, 0:N],
                                     brow_sb[0:1, idx * 128:(idx + 1) * 128],
                                     ones_sb,
                                     start=False, stop=True, skip_group_check=True)
                nc.tensor.matmul(n_ps[:, di, 0:N],
                                 brow_sb[0:1, (4 + di) * 128:(5 + di) * 128],
                                 ones_sb, start=True, stop=True)
                xn_ps = p_e.tile([128, N], F32, tag="escratch")
                nc.tensor.matmul(xn_ps, w_sb[:, di, 256:384], rhs_e,
                                 start=True, stop=True)
                nc.scalar.add(xn_sb[:, di, :], xn_ps, bihn_sb[:, di:di + 1])
        cm_e.__exit__(None, None, None)
        if lvl == 2:
            dump(xn_sb[0:BPC, 0, :], N)

        if lvl >= 3:
            nc.vector.memset(HS[:, 0], 0.0)
            # ---- phase 3: GRU scan ----
            for t in range(T):
                hf = HS[:, t, 0, :]
                hb = HS[:, t, 1, :]
                c0, c1 = t * BPC, (t + 1) * BPC
                nc.tensor.matmul(rz_ps[:, 0, 0, c0:c1], w_sb[:, 2, 0:128], hf,
                                 start=False, stop=True, skip_group_check=True)
                nc.tensor.matmul(rz_ps[:, 0, 1, c0:c1], w_sb[:, 2, 128:256], hf,
                                 start=False, stop=True, skip_group_check=True)
                nc.tensor.matmul(rz_ps[:, 1, 0, c0:c1], w_sb[:, 3, 0:128], hb,
                                 start=False, stop=True, skip_group_check=True)
                nc.tensor.matmul(rz_ps[:, 1, 1, c0:c1], w_sb[:, 3, 128:256], hb,
                                 start=False, stop=True, skip_group_check=True)
                nc.tensor.matmul(n_ps[:, 0, c0:c1], w_sb[:, 2, 256:384], hf,
                                 start=False, stop=True, skip_group_check=True)
                nc.tensor.matmul(n_ps[:, 1, c0:c1], w_sb[:, 3, 256:384], hb,
                                 start=False, stop=True, skip_group_check=True)

                sig = sb_scan.tile([128, 2, 2, BPC], F32)
                nc.scalar.activation(sig, rz_ps[:, :, :, c0:c1], AF.Sigmoid)
                rn = sb_scan.tile([128, 2, BPC], F32)
                nc.vector.tensor_mul(rn, sig[:, :, 0, :], n_ps[:, :, c0:c1])
                arg = sb_scan.tile([128, 2, BPC], F32)
                nc.vector.tensor_add(arg, rn, xn_sb[:, :, c0:c1])
                nt = sb_scan.tile([128, 2, BPC], F32)
                nc.scalar.activation(nt, arg, AF.Tanh)
                dd = sb_scan.tile([128, 2, BPC], F32)
                nc.vector.tensor_sub(dd, HS[:, t], nt)
                zd = sb_scan.tile([128, 2, BPC], F32)
                nc.vector.tensor_mul(zd, sig[:, :, 1, :], dd)
                nc.vector.tensor_add(HS[:, t + 1], zd, nt)
                nc.gpsimd.tensor_copy(HSb[:, T - 1 - t, :], HS[:, t + 1, 1, :])
        cm_x3.__exit__(None, None, None)
        if lvl == 3:
            dump(HSb[0:BPC, 0:50, :], 50 * BPC)

        if lvl >= 4:
            # ---- phase 4: attention + head ----
            p_a = ctx.enter_context(
                tc.tile_pool(name="p_a", bufs=1, space="PSUM"))
            hf_view = HS[:, 1:T + 1, 0, :]                 # [128, 100, 4]
            hb_view = HSb[:, :, :]

            abs_e = sb_m.tile([128, N], F32)
            nc.vector.tensor_mul(abs_e, e_sb, e_sb)
            sa_ps = p_a.tile([1, N], F32)
            nc.tensor.matmul(sa_ps, onescol_sb, abs_e, start=True, stop=True)
            s_ps = p_a.tile([1, N], F32)
            nc.tensor.matmul(s_ps, attnw_sb[:, 0:1], hf_view,
                             start=True, stop=False)
            nc.tensor.matmul(s_ps, attnw_sb[:, 1:2], hb_view,
                             start=False, stop=True)

            stage1 = sb_m.tile([1, 2, N], F32)
            nc.scalar.copy(stage1[:, 0, :], s_ps)
            nc.scalar.copy(stage1[:, 1, :], sa_ps)
            nc.sync.dma_start(out=d_b1, in_=stage1)
            s4 = sb_m.tile([4, 2, T], F32)
            # linear idx of (w, t, b) in bounce1 = w*400 + t*4 + b
            nc.sync.dma_start(
                out=s4, in_=d_b1.rearrange("(w t b) -> b w t", w=2, t=T, b=4))
            if lvl == 4:
                dump(s4[:, :, :], 2 * T)

        if lvl >= 5:
            pen4 = sb_m.tile([4, T], F32)
            nc.vector.tensor_scalar(pen4, s4[:, 1, :], 0.0, -1e9,
                                    ALU.is_equal, ALU.mult)
            m4 = sb_m.tile([4, T], F32)
            k4 = sb_m.tile([4, 1], F32)
            nc.vector.tensor_scalar(m4, s4[:, 1, :], 0.0, None, ALU.is_gt,
                                    op1=ALU.add, accum_out=k4)
            sm4 = sb_m.tile([4, T], F32)
            nc.vector.scalar_tensor_tensor(sm4, s4[:, 0, :], attnb_sb, pen4,
                                           ALU.add, ALU.add)
            negmax = sb_m.tile([4, 1], F32)
            nc.vector.reduce_max(negmax, sm4, AX.X, negate=True)
            easel = sb_m.tile([4, 2, T], F32)
            esum = sb_m.tile([4, 1], F32)
            nc.scalar.activation(easel[:, 0, :], sm4, AF.Exp, bias=negmax,
                                 accum_out=esum)
            rcp = sb_m.tile([4, 1], F32)
            nc.vector.reciprocal(rcp, esum)
            nc.vector.tensor_scalar_mul(easel[:, 0, :], easel[:, 0, :], rcp)
            nc.vector.tensor_scalar(easel[:, 1, :], iota_sb, k4, None,
                                    ALU.is_equal)
            nc.sync.dma_start(out=d_b2, in_=easel)
            flat2 = sb_m.tile([1, 2, T, 4], F32)
            # linear idx of (b, w, t) in bounce2 = b*200 + w*100 + t
            nc.sync.dma_start(
                out=flat2, in_=d_b2.rearrange("(b w t) -> w t b", b=4, w=2, t=T))
            if lvl == 5:
                dbg5 = sb_m.tile([BPC, OUT], F32)
                nc.vector.memset(dbg5, 0.0)
                nc.vector.tensor_copy(dbg5[0:1, 0:800], flat2[0:1, :, :, :])
                nc.sync.dma_start(out=d_out, in_=dbg5)

        if lvl >= 6:
            aB = p_a.tile([128, T, 4], F32)
            nc.tensor.matmul(aB, ones_sb[0:1, 0:128], flat2[:, 0],
                             start=True, stop=True)
            selB = p_a.tile([128, T, 4], F32)
            nc.tensor.matmul(selB, ones_sb[0:1, 0:128], flat2[:, 1],
                             start=True, stop=True)

            if upto == "ab":
                abd = sb_m.tile([BPC, OUT], F32)
                nc.vector.memset(abd, 0.0)
                nc.vector.tensor_copy(abd[:, 0:N], aB[0:BPC, :, :])
                nc.vector.tensor_copy(abd[:, N:2 * N], selB[0:BPC, :, :])
                nc.sync.dma_start(out=d_out, in_=abd)
            cc_sb = None if upto == "ab" else sb_m.tile([128, 4, BPC], F32)
            for blk, (wps, hview) in enumerate([] if upto == "ab" else
                    [(aB, hf_view), (aB, hb_view),
                     (selB, hf_view), (selB, hb_view)]):
                tmp = sb_scan.tile([128, T, BPC], F32, tag="ctx_tmp")
                nc.vector.tensor_mul(tmp, hview, wps)
                nc.vector.tensor_reduce(
                    cc_sb[:, blk, :], tmp.rearrange("p t b -> p b t"),
                    AX.X, ALU.add)

            if lvl == 6 and upto != "ab":
                dump(cc_sb[0:BPC, :, :], 16)

        if lvl >= 7:
            feat_ps = p_a.tile([128, BPC], F32)
            for i in range(4):
                nc.tensor.matmul(feat_ps, combw_sb[:, i * 128:(i + 1) * 128],
                                 cc_sb[:, i, :], start=(i == 0), stop=(i == 3))
            featT = sb_m.tile([128, BPC], F32)
            nc.scalar.activation(featT, feat_ps, AF.Tanh, bias=combb_sb)
            if lvl == 7:
                dump(featT[0:BPC, :], BPC)

        if lvl >= 8:
            lg0 = p_a.tile([BPC, 512], F32)
            nc.tensor.matmul(lg0, featT, fcw_sb[:, 0:512],
                             start=True, stop=False)
            nc.tensor.matmul(lg0, ones_sb[0:1, 0:BPC], fcb_sb[0:1, 0:512],
                             start=False, stop=True)
            lg1 = p_a.tile([BPC, OUT - 512], F32)
            nc.tensor.matmul(lg1, featT, fcw_sb[:, 512:OUT],
                             start=True, stop=False)
            nc.tensor.matmul(lg1, ones_sb[0:1, 0:BPC], fcb_sb[0:1, 512:OUT],
                             start=False, stop=True)
            out_sb = sb_m.tile([BPC, OUT], F32)
            nc.scalar.copy(out_sb[:, 0:512], lg0)
            nc.scalar.copy(out_sb[:, 512:OUT], lg1)
            nc.sync.dma_start(out=d_out, in_=out_sb)

    nc.compile()
    return nc


def prep_inputs(batchdata, emb, wih_f, whh_f, bih_f, bhh_f, wih_b, whh_b,
                bih_b, bhh_b, attn_w, attn_b, comb_w, comb_b, fc_w, fc_b):
    """Host-side sharding + layout prep. Returns per-core in_maps."""
    f32 = np.float32
    batchdata = np.asarray(batchdata, f32)
    emb = np.asarray(emb, f32)

    # emb hi/lo split, padded to VP rows, tiled [KT, 128, 256]
    hi = emb.astype(bf16)
    lo = (emb - hi.astype(f32)).astype(bf16)
    embhl = np.zeros((KT, 128, 256), bf16)
    hl = np.concatenate([hi, lo], axis=1)          # [V, 256]
    embhl.reshape(VP, 256)[:V] = hl

    def t_(a):
        return np.ascontiguousarray(np.asarray(a, f32).T)

    shared = {
        "embhl": embhl,
        "wihT_f": t_(wih_f), "wihT_b": t_(wih_b),
        "whhT_f": t_(whh_f), "whhT_b": t_(whh_b),
        "bias_rows": np.concatenate([
            (np.asarray(bih_f, f32) + np.asarray(bhh_f, f32))[0:256],
            (np.asarray(bih_b, f32) + np.asarray(bhh_b, f32))[0:256],
            np.asarray(bhh_f, f32)[256:384],
            np.asarray(bhh_b, f32)[256:384],
        ]).reshape(1, 768).copy(),
        "bihn": np.stack([np.asarray(bih_f, f32)[256:384],
                          np.asarray(bih_b, f32)[256:384]], axis=1).copy(),
        "attn_wc": np.ascontiguousarray(
            np.asarray(attn_w, f32).reshape(2, 128, 1)[:, :, 0].T),
        "attn_b4": np.full((4, 1), np.asarray(attn_b, f32).reshape(-1)[0], f32),
        "comb_wT": np.ascontiguousarray(
            t_(comb_w).reshape(4, 128, 128).transpose(1, 0, 2).reshape(128, 512)),
        "comb_b": np.asarray(comb_b, f32).reshape(128, 1).copy(),
        "fc_wT": t_(fc_w),
        "fc_b": np.asarray(fc_b, f32).reshape(1, OUT).copy(),
        "iota4": np.broadcast_to(
            np.arange(1, T + 1, dtype=f32)[None, :], (4, T)).copy(),
    }

    in_maps = []
    for c in range(NCORES):
        xc = batchdata[c * BPC:(c + 1) * BPC]       # [4, 100, V]
        # columns time-major: col j = t*BPC + b  ->  rows of [T, BPC, V]
        x2 = np.ascontiguousarray(
            xc.transpose(1, 0, 2).reshape(N, V).T.astype(bf16))  # [V, N]
        xt = np.zeros((KT, 128, N), bf16)
        xt.reshape(VP, N)[:V] = x2
        in_maps.append({"xt": xt, **shared})
    return in_maps


_NC_CACHE = {}


def get_compiled():
    if "nc" not in _NC_CACHE:
        nc = build_nc()
        nc.m = get_hw_module(nc.m)
        _NC_CACHE["nc"] = nc
    return _NC_CACHE["nc"]


def kernel(**inputs):
    nc = get_compiled()
    in_maps = prep_inputs(**inputs)
    res = bass_utils.run_bass_kernel_spmd(
        nc, in_maps, core_ids=list(range(NCORES)))
    out = np.concatenate([res.results[c]["logits"] for c in range(NCORES)],
                         axis=0)
    return out.astype(np.float32)


# revision 20
# speedup vs baseline: 1.6375x; 1.6375x over previous
"""Trainium2 Bass kernel for nn_Dipole (multi-hot embedding + BiGRU + attention + FC).

Self-contained: hardcodes shapes B=32, T=100, V=10000, D=128, OUT=1000, 8 cores.
Sharding: data-parallel over batch (4 patients per core); weights replicated.

Per-core pipeline (all layouts d-major [128, ...], time-major free columns):
  1. e.T accumulated in fp32 PSUM from bf16 operands (batchdata is exactly
     representable in bf16; emb is split hi+lo bf16 so the pair sums to the
     fp32 value within ~2^-17 relative).
  2. x3 = e @ wih.T + (all foldable biases), computed fp32 and LEFT RESIDENT
     in PSUM banks; backward direction stored time-reversed so both
     directions share column blocks during the scan.
  3. GRU scan: per tick, 6 small fp32 matmuls accumulate whh @ h directly
     onto the x3 PSUM columns (has_written accumulate), one fused sigmoid
     over r/z x both dirs, one tanh, 5 DVE ops - all [128, 2*4] tiles.
  4. Attention scores via matmul, segmented softmax in [4,100] layout (DRAM
     bounce for partition reshape), masking + last-index selection, context
     via K=1 broadcast matmuls + tensor_tensor_reduce, then comb/fc matmuls.
"""

import sys

sys.path.insert(0, "/opt/trn_rl_repo")

import numpy as np
import ml_dtypes

import concourse.bass as bass
import concourse.bacc as bacc
import concourse.tile as tile
from concourse import mybir
from concourse import bass_utils
from concourse.bass_interp import get_hw_module

F32 = mybir.dt.float32
BF16 = mybir.dt.bfloat16
AF = mybir.ActivationFunctionType
ALU = mybir.AluOpType
AX = mybir.AxisListType

B, T, V, D, OUT = 32, 100, 10000, 128, 1000
NCORES = 8
BPC = B // NCORES          # 4 patients per core
N = BPC * T                # 400 free columns (time-major: col = t*BPC + b)
KT = (V + 127) // 128      # 79 k-tiles
VP = KT * 128              # 10112 padded vocab
KB = 8                     # k-tiles per DMA batch
NKB = (KT + KB - 1) // KB

bf16 = ml_dtypes.bfloat16

_STAGES = {"e": 1, "x3": 2, "scan": 3, "scores": 4, "soft": 5, "ab": 6, "ctx": 6, "feat": 7, "full": 9}


def build_nc(upto="full"):
    """Emit + compile the Bass program (single NEFF, SPMD across 8 cores).
    upto < full truncates after that stage and dumps a debug view."""
    lvl = _STAGES[upto]
    nc = bacc.Bacc("TRN2", target_bir_lowering=False, debug=False,
                   enable_asserts=False)

    # ---- DRAM I/O ----
    d_xt = nc.dram_tensor("xt", [KT, 128, N], BF16, kind="ExternalInput").ap()
    d_emb = nc.dram_tensor("embhl", [KT, 128, 256], BF16, kind="ExternalInput").ap()
    d_wih_f = nc.dram_tensor("wihT_f", [128, 384], F32, kind="ExternalInput").ap()
    d_wih_b = nc.dram_tensor("wihT_b", [128, 384], F32, kind="ExternalInput").ap()
    d_whh_f = nc.dram_tensor("whhT_f16", [128, 384], BF16, kind="ExternalInput").ap()
    d_whh_b = nc.dram_tensor("whhT_b16", [128, 384], BF16, kind="ExternalInput").ap()
    d_brow = nc.dram_tensor("bias_rows", [1, 768], F32, kind="ExternalInput").ap()
    d_bihn = nc.dram_tensor("bihn", [128, 2], F32, kind="ExternalInput").ap()
    d_attnw = nc.dram_tensor("attn_wc", [128, 2], F32, kind="ExternalInput").ap()
    d_attnb = nc.dram_tensor("attn_b4", [4, 1], F32, kind="ExternalInput").ap()
    d_combw = nc.dram_tensor("comb_wT", [128, 512], F32, kind="ExternalInput").ap()
    d_combb = nc.dram_tensor("comb_b", [128, 1], F32, kind="ExternalInput").ap()
    d_fcw = nc.dram_tensor("fc_wT", [128, OUT], F32, kind="ExternalInput").ap()
    d_fcb = nc.dram_tensor("fc_b", [1, OUT], F32, kind="ExternalInput").ap()
    d_iota = nc.dram_tensor("iota4", [4, T], F32, kind="ExternalInput").ap()
    d_out = nc.dram_tensor("logits", [BPC, OUT], F32, kind="ExternalOutput").ap()
    d_b1 = nc.dram_tensor("bounce1", [2 * N], F32, kind="Internal").ap()
    d_b2 = nc.dram_tensor("bounce2", [2 * N], F32, kind="Internal").ap()

    from contextlib import ExitStack
    with tile.TileContext(nc) as tc, ExitStack() as ctx:
        cm_x3 = tc.tile_pool(name="p_x3", bufs=1, space="PSUM")
        p_x3 = cm_x3.__enter__()
        cm_e = tc.tile_pool(name="p_e", bufs=2, space="PSUM")
        p_e = cm_e.__enter__()
        sb_c = ctx.enter_context(tc.tile_pool(name="sb_c", bufs=1))
        sb_m = ctx.enter_context(tc.tile_pool(name="sb_m", bufs=1))
        sb_scan = ctx.enter_context(tc.tile_pool(name="sb_scan", bufs=2))
        sb_x = ctx.enter_context(tc.tile_pool(name="sb_x", bufs=3))
        sb_e = ctx.enter_context(tc.tile_pool(name="sb_e", bufs=3))

        # ---- constants into SBUF ----
        w_sb = sb_c.tile([128, 2, 384], F32)       # wihf, wihb
        nc.sync.dma_start(out=w_sb[:, 0, :], in_=d_wih_f)
        nc.sync.dma_start(out=w_sb[:, 1, :], in_=d_wih_b)
        w16_sb = sb_c.tile([128, 2, 384], BF16)    # whhf, whhb (scan weights)
        nc.sync.dma_start(out=w16_sb[:, 0, :], in_=d_whh_f)
        nc.sync.dma_start(out=w16_sb[:, 1, :], in_=d_whh_b)
        brow_sb = sb_c.tile([1, 768], F32)
        nc.sync.dma_start(out=brow_sb, in_=d_brow)
        bihn_sb = sb_c.tile([128, 2], F32)
        nc.sync.dma_start(out=bihn_sb, in_=d_bihn)
        attnw_sb = sb_c.tile([128, 2], F32)
        nc.sync.dma_start(out=attnw_sb, in_=d_attnw)
        attnb_sb = sb_c.tile([4, 1], F32)
        nc.sync.dma_start(out=attnb_sb, in_=d_attnb)
        combw_sb = sb_c.tile([128, 512], F32)
        nc.sync.dma_start(out=combw_sb, in_=d_combw)
        combb_sb = sb_c.tile([128, 1], F32)
        nc.sync.dma_start(out=combb_sb, in_=d_combb)
        fcw_sb = sb_c.tile([128, OUT], F32)
        nc.sync.dma_start(out=fcw_sb, in_=d_fcw)
        fcb_sb = sb_c.tile([1, OUT], F32)
        nc.sync.dma_start(out=fcb_sb, in_=d_fcb)
        iota_sb = sb_c.tile([4, T], F32)
        nc.sync.dma_start(out=iota_sb, in_=d_iota)
        ones_sb = sb_c.tile([1, N], F32)
        nc.vector.memset(ones_sb, 1.0)
        onescol_sb = sb_c.tile([128, 1], F32)
        nc.vector.memset(onescol_sb, 1.0)

        # ---- long-lived SBUF state ----
        e_sb = sb_m.tile([128, N], F32)            # e.T, col = t*BPC + b
        xn_sb = sb_m.tile([128, 2, N], F32)        # xn + bih_n; dir b time-reversed
        HS = sb_m.tile([128, T + 1, 2, BPC], F32)  # [.,tau,0,.]=hf(tau-1), [.,tau,1,.]=hb(T-tau)
        HSb = sb_m.tile([128, T, BPC], F32)        # hb in true time order
        HC = sb_m.tile([128, T + 1, 2, BPC], BF16)  # bf16 mirror of HS for PE

        def dump(src_ap, nfree):
            dbg = sb_m.tile([BPC, OUT], F32)
            nc.vector.memset(dbg, 0.0)
            nc.vector.tensor_copy(dbg[:, 0:nfree], src_ap)
            nc.sync.dma_start(out=d_out, in_=dbg)

        # ---- phase 1: e.T accumulation in PSUM ----
        e_ps = p_e.tile([128, N], F32, tag="escratch")
        for kb in range(NKB):
            nk = min(KB, KT - kb * KB)
            xk = sb_x.tile([128, KB, N], BF16)
            nc.sync.dma_start(
                out=xk[:, :nk, :],
                in_=d_xt[kb * KB:kb * KB + nk].rearrange("k p n -> p k n"))
            ek = sb_e.tile([128, KB, 256], BF16)
            nc.sync.dma_start(
                out=ek[:, :nk, :],
                in_=d_emb[kb * KB:kb * KB + nk].rearrange("k p n -> p k n"))
            for j in range(nk):
                k = kb * KB + j
                nc.tensor.matmul(e_ps, ek[:, j, 0:128], xk[:, j, :],
                                 start=(k == 0), stop=False)
                nc.tensor.matmul(e_ps, ek[:, j, 128:256], xk[:, j, :],
                                 start=False, stop=(k == KT - 1))
        nc.scalar.copy(e_sb, e_ps)
        if lvl == 1:
            dump(e_sb[0:BPC, :], N)

        if lvl >= 2:
            # reversed-time copy of e.T (hardware rejects negative-stride
            # APs; materialize the reversal with per-block copies)
            e_rev = sb_m.tile([128, N], F32)
            for tt in range(T):
                src_blk = e_sb[:, (T - 1 - tt) * BPC:(T - tt) * BPC]
                dst_blk = e_rev[:, tt * BPC:(tt + 1) * BPC]
                if tt % 2 == 0:
                    nc.vector.tensor_copy(dst_blk, src_blk)
                else:
                    nc.gpsimd.tensor_copy(dst_blk, src_blk)

            # ---- phase 2: x3 resident in PSUM + xn in SBUF ----
            rz_ps = p_x3.tile([128, 2, 2, 512], F32)   # [dir][gate r,z]
            n_ps = p_x3.tile([128, 2, 512], F32)       # [dir]
            for di in range(2):
                rhs_e = e_sb if di == 0 else e_rev
                for g in range(2):  # r, z
                    nc.tensor.matmul(rz_ps[:, di, g, 0:N],
                                     w_sb[:, di, g * 128:(g + 1) * 128], rhs_e,
                                     start=True, stop=True)
                    idx = di * 2 + g
                    nc.tensor.matmul(rz_ps[:, di, g, 0:N],
                                     brow_sb[0:1, idx * 128:(idx + 1) * 128],
                                     ones_sb,
                                     start=False, stop=True, skip_group_check=True)
                nc.tensor.matmul(n_ps[:, di, 0:N],
                                 brow_sb[0:1, (4 + di) * 128:(5 + di) * 128],
                                 ones_sb, start=True, stop=True)
                xn_ps = p_e.tile([128, N], F32, tag="escratch")
                nc.tensor.matmul(xn_ps, w_sb[:, di, 256:384], rhs_e,
                                 start=True, stop=True)
                nc.scalar.add(xn_sb[:, di, :], xn_ps, bihn_sb[:, di:di + 1])
        cm_e.__exit__(None, None, None)
        if lvl == 2:
            dump(xn_sb[0:BPC, 0, :], N)

        if lvl >= 3:
            nc.vector.memset(HS[:, 0], 0.0)
            nc.vector.memset(HC[:, 0], 0.0)
            # ---- phase 3: GRU scan ----
            for t in range(T):
                hf = HC[:, t, 0, :]
                hb = HC[:, t, 1, :]
                c0, c1 = t * BPC, (t + 1) * BPC
                nc.tensor.matmul(rz_ps[:, 0, 0, c0:c1], w16_sb[:, 0, 0:128], hf,
                                 start=False, stop=True, skip_group_check=True)
                nc.tensor.matmul(rz_ps[:, 0, 1, c0:c1], w16_sb[:, 0, 128:256], hf,
                                 start=False, stop=True, skip_group_check=True)
                nc.tensor.matmul(rz_ps[:, 1, 0, c0:c1], w16_sb[:, 1, 0:128], hb,
                                 start=False, stop=True, skip_group_check=True)
                nc.tensor.matmul(rz_ps[:, 1, 1, c0:c1], w16_sb[:, 1, 128:256], hb,
                                 start=False, stop=True, skip_group_check=True)
                nc.tensor.matmul(n_ps[:, 0, c0:c1], w16_sb[:, 0, 256:384], hf,
                                 start=False, stop=True, skip_group_check=True)
                nc.tensor.matmul(n_ps[:, 1, c0:c1], w16_sb[:, 1, 256:384], hb,
                                 start=False, stop=True, skip_group_check=True)

                sig = sb_scan.tile([128, 2, 2, BPC], F32)
                nc.scalar.activation(sig, rz_ps[:, :, :, c0:c1], AF.Sigmoid)
                rn = sb_scan.tile([128, 2, BPC], F32)
                nc.vector.tensor_mul(rn, sig[:, :, 0, :], n_ps[:, :, c0:c1])
                arg = sb_scan.tile([128, 2, BPC], F32)
                nc.vector.tensor_add(arg, rn, xn_sb[:, :, c0:c1])
                nt = sb_scan.tile([128, 2, BPC], F32)
                nc.scalar.activation(nt, arg, AF.Tanh)
                dd = sb_scan.tile([128, 2, BPC], F32)
                nc.vector.tensor_sub(dd, HS[:, t], nt)
                zd = sb_scan.tile([128, 2, BPC], F32)
                nc.vector.tensor_mul(zd, sig[:, :, 1, :], dd)
                nc.vector.tensor_add(HS[:, t + 1], zd, nt)
                nc.vector.tensor_add(HC[:, t + 1], zd, nt)
                nc.gpsimd.tensor_copy(HSb[:, T - 1 - t, :], HS[:, t + 1, 1, :])
        cm_x3.__exit__(None, None, None)
        if lvl == 3:
            dump(HSb[0:BPC, 0:50, :], 50 * BPC)

        if lvl >= 4:
            # ---- phase 4: attention + head ----
            p_a = ctx.enter_context(
                tc.tile_pool(name="p_a", bufs=1, space="PSUM"))
            hf_view = HS[:, 1:T + 1, 0, :]                 # [128, 100, 4]
            hb_view = HSb[:, :, :]

            abs_e = sb_m.tile([128, N], F32)
            nc.vector.tensor_mul(abs_e, e_sb, e_sb)
            sa_ps = p_a.tile([1, N], F32)
            nc.tensor.matmul(sa_ps, onescol_sb, abs_e, start=True, stop=True)
            s_ps = p_a.tile([1, N], F32)
            nc.tensor.matmul(s_ps, attnw_sb[:, 0:1], hf_view,
                             start=True, stop=False)
            nc.tensor.matmul(s_ps, attnw_sb[:, 1:2], hb_view,
                             start=False, stop=True)

            stage1 = sb_m.tile([1, 2, N], F32)
            nc.scalar.copy(stage1[:, 0, :], s_ps)
            nc.scalar.copy(stage1[:, 1, :], sa_ps)
            nc.sync.dma_start(out=d_b1, in_=stage1)
            s4 = sb_m.tile([4, 2, T], F32)
            # linear idx of (w, t, b) in bounce1 = w*400 + t*4 + b
            nc.sync.dma_start(
                out=s4, in_=d_b1.rearrange("(w t b) -> b w t", w=2, t=T, b=4))
            if lvl == 4:
                dump(s4[:, :, :], 2 * T)

        if lvl >= 5:
            pen4 = sb_m.tile([4, T], F32)
            nc.vector.tensor_scalar(pen4, s4[:, 1, :], 0.0, -1e9,
                                    ALU.is_equal, ALU.mult)
            m4 = sb_m.tile([4, T], F32)
            k4 = sb_m.tile([4, 1], F32)
            nc.vector.tensor_scalar(m4, s4[:, 1, :], 0.0, None, ALU.is_gt,
                                    op1=ALU.add, accum_out=k4)
            sm4 = sb_m.tile([4, T], F32)
            nc.vector.scalar_tensor_tensor(sm4, s4[:, 0, :], attnb_sb, pen4,
                                           ALU.add, ALU.add)
            negmax = sb_m.tile([4, 1], F32)
            nc.vector.reduce_max(negmax, sm4, AX.X, negate=True)
            easel = sb_m.tile([4, 2, T], F32)
            esum = sb_m.tile([4, 1], F32)
            nc.scalar.activation(easel[:, 0, :], sm4, AF.Exp, bias=negmax,
                                 accum_out=esum)
            rcp = sb_m.tile([4, 1], F32)
            nc.vector.reciprocal(rcp, esum)
            nc.vector.tensor_scalar_mul(easel[:, 0, :], easel[:, 0, :], rcp)
            nc.vector.tensor_scalar(easel[:, 1, :], iota_sb, k4, None,
                                    ALU.is_equal)
            nc.sync.dma_start(out=d_b2, in_=easel)
            flat2 = sb_m.tile([1, 2, T, 4], F32)
            # linear idx of (b, w, t) in bounce2 = b*200 + w*100 + t
            nc.sync.dma_start(
                out=flat2, in_=d_b2.rearrange("(b w t) -> w t b", b=4, w=2, t=T))
            if lvl == 5:
                dbg5 = sb_m.tile([BPC, OUT], F32)
                nc.vector.memset(dbg5, 0.0)
                nc.vector.tensor_copy(dbg5[0:1, 0:800], flat2[0:1, :, :, :])
                nc.sync.dma_start(out=d_out, in_=dbg5)

        if lvl >= 6:
            aB = p_a.tile([128, T, 4], F32)
            nc.tensor.matmul(aB, ones_sb[0:1, 0:128], flat2[:, 0],
                             start=True, stop=True)
            selB = p_a.tile([128, T, 4], F32)
            nc.tensor.matmul(selB, ones_sb[0:1, 0:128], flat2[:, 1],
                             start=True, stop=True)

            if upto == "ab":
                abd = sb_m.tile([BPC, OUT], F32)
                nc.vector.memset(abd, 0.0)
                nc.vector.tensor_copy(abd[:, 0:N], aB[0:BPC, :, :])
                nc.vector.tensor_copy(abd[:, N:2 * N], selB[0:BPC, :, :])
                nc.sync.dma_start(out=d_out, in_=abd)
            cc_sb = None if upto == "ab" else sb_m.tile([128, 4, BPC], F32)
            for blk, (wps, hview) in enumerate([] if upto == "ab" else
                    [(aB, hf_view), (aB, hb_view),
                     (selB, hf_view), (selB, hb_view)]):
                tmp = sb_scan.tile([128, T, BPC], F32, tag="ctx_tmp")
                nc.vector.tensor_mul(tmp, hview, wps)
                nc.vector.tensor_reduce(
                    cc_sb[:, blk, :], tmp.rearrange("p t b -> p b t"),
                    AX.X, ALU.add)

            if lvl == 6 and upto != "ab":
                dump(cc_sb[0:BPC, :, :], 16)

        if lvl >= 7:
            feat_ps = p_a.tile([128, BPC], F32)
            for i in range(4):
                nc.tensor.matmul(feat_ps, combw_sb[:, i * 128:(i + 1) * 128],
                                 cc_sb[:, i, :], start=(i == 0), stop=(i == 3))
            featT = sb_m.tile([128, BPC], F32)
            nc.scalar.activation(featT, feat_ps, AF.Tanh, bias=combb_sb)
            if lvl == 7:
                dump(featT[0:BPC, :], BPC)

        if lvl >= 8:
            lg0 = p_a.tile([BPC, 512], F32)
            nc.tensor.matmul(lg0, featT, fcw_sb[:, 0:512],
                             start=True, stop=False)
            nc.tensor.matmul(lg0, ones_sb[0:1, 0:BPC], fcb_sb[0:1, 0:512],
                             start=False, stop=True)
            lg1 = p_a.tile([BPC, OUT - 512], F32)
            nc.tensor.matmul(lg1, featT, fcw_sb[:, 512:OUT],
                             start=True, stop=False)
            nc.tensor.matmul(lg1, ones_sb[0:1, 0:BPC], fcb_sb[0:1, 512:OUT],
                             start=False, stop=True)
            out_sb = sb_m.tile([BPC, OUT], F32)
            nc.scalar.copy(out_sb[:, 0:512], lg0)
            nc.scalar.copy(out_sb[:, 512:OUT], lg1)
            nc.sync.dma_start(out=d_out, in_=out_sb)

    nc.compile()
    return nc


def prep_inputs(batchdata, emb, wih_f, whh_f, bih_f, bhh_f, wih_b, whh_b,
                bih_b, bhh_b, attn_w, attn_b, comb_w, comb_b, fc_w, fc_b):
    """Host-side sharding + layout prep. Returns per-core in_maps."""
    f32 = np.float32
    batchdata = np.asarray(batchdata, f32)
    emb = np.asarray(emb, f32)

    # emb hi/lo split, padded to VP rows, tiled [KT, 128, 256]
    hi = emb.astype(bf16)
    lo = (emb - hi.astype(f32)).astype(bf16)
    embhl = np.zeros((KT, 128, 256), bf16)
    hl = np.concatenate([hi, lo], axis=1)          # [V, 256]
    embhl.reshape(VP, 256)[:V] = hl

    def t_(a):
        return np.ascontiguousarray(np.asarray(a, f32).T)

    shared = {
        "embhl": embhl,
        "wihT_f": t_(wih_f), "wihT_b": t_(wih_b),
        "whhT_f16": t_(whh_f).astype(bf16), "whhT_b16": t_(whh_b).astype(bf16),
        "bias_rows": np.concatenate([
            (np.asarray(bih_f, f32) + np.asarray(bhh_f, f32))[0:256],
            (np.asarray(bih_b, f32) + np.asarray(bhh_b, f32))[0:256],
            np.asarray(bhh_f, f32)[256:384],
            np.asarray(bhh_b, f32)[256:384],
        ]).reshape(1, 768).copy(),
        "bihn": np.stack([np.asarray(bih_f, f32)[256:384],
                          np.asarray(bih_b, f32)[256:384]], axis=1).copy(),
        "attn_wc": np.ascontiguousarray(
            np.asarray(attn_w, f32).reshape(2, 128, 1)[:, :, 0].T),
        "attn_b4": np.full((4, 1), np.asarray(attn_b, f32).reshape(-1)[0], f32),
        "comb_wT": np.ascontiguousarray(
            t_(comb_w).reshape(4, 128, 128).transpose(1, 0, 2).reshape(128, 512)),
        "comb_b": np.asarray(comb_b, f32).reshape(128, 1).copy(),
        "fc_wT": t_(fc_w),
        "fc_b": np.asarray(fc_b, f32).reshape(1, OUT).copy(),
        "iota4": np.broadcast_to(
            np.arange(1, T + 1, dtype=f32)[None, :], (4, T)).copy(),
    }

    in_maps = []
    for c in range(NCORES):
        xc = batchdata[c * BPC:(c + 1) * BPC]       # [4, 100, V]
        # columns time-major: col j = t*BPC + b  ->  rows of [T, BPC, V]
        x2 = np.ascontiguousarray(
            xc.transpose(1, 0, 2).reshape(N, V).T.astype(bf16))  # [V, N]
        xt = np.zeros((KT, 128, N), bf16)
        xt.reshape(VP, N)[:V] = x2
        in_maps.append({"xt": xt, **shared})
    return in_maps


_NC_CACHE = {}


def get_compiled():
    if "nc" not in _NC_CACHE:
        nc = build_nc()
        nc.m = get_hw_module(nc.m)
        _NC_CACHE["nc"] = nc
    return _NC_CACHE["nc"]


def kernel(**inputs):
    nc = get_compiled()
    in_maps = prep_inputs(**inputs)
    res = bass_utils.run_bass_kernel_spmd(
        nc, in_maps, core_ids=list(range(NCORES)))
    out = np.concatenate([res.results[c]["logits"] for c in range(NCORES)],
                         axis=0)
    return out.astype(np.float32)


# revision 22
# speedup vs baseline: 1.7272x; 1.0548x over previous
"""Trainium2 Bass kernel for nn_Dipole (multi-hot embedding + BiGRU + attention + FC).

Self-contained: hardcodes shapes B=32, T=100, V=10000, D=128, OUT=1000, 8 cores.
Sharding: data-parallel over batch (4 patients per core); weights replicated.

Per-core pipeline (all layouts d-major [128, ...], time-major free columns):
  1. e.T accumulated in fp32 PSUM from bf16 operands (batchdata is exactly
     representable in bf16; emb is split hi+lo bf16 so the pair sums to the
     fp32 value within ~2^-17 relative).
  2. x3 = e @ wih.T + (all foldable biases), computed fp32 and LEFT RESIDENT
     in PSUM banks; backward direction stored time-reversed so both
     directions share column blocks during the scan.
  3. GRU scan: per tick, 6 small fp32 matmuls accumulate whh @ h directly
     onto the x3 PSUM columns (has_written accumulate), one fused sigmoid
     over r/z x both dirs, one tanh, 5 DVE ops - all [128, 2*4] tiles.
  4. Attention scores via matmul, segmented softmax in [4,100] layout (DRAM
     bounce for partition reshape), masking + last-index selection, context
     via K=1 broadcast matmuls + tensor_tensor_reduce, then comb/fc matmuls.
"""

import sys

sys.path.insert(0, "/opt/trn_rl_repo")

import numpy as np
import ml_dtypes

import concourse.bass as bass
import concourse.bacc as bacc
import concourse.tile as tile
from concourse import mybir
from concourse import bass_utils
from concourse.bass_interp import get_hw_module

F32 = mybir.dt.float32
BF16 = mybir.dt.bfloat16
AF = mybir.ActivationFunctionType
ALU = mybir.AluOpType
AX = mybir.AxisListType

B, T, V, D, OUT = 32, 100, 10000, 128, 1000
NCORES = 8
BPC = B // NCORES          # 4 patients per core
N = BPC * T                # 400 free columns (time-major: col = t*BPC + b)
KT = (V + 127) // 128      # 79 k-tiles
VP = KT * 128              # 10112 padded vocab
KB = 8                     # k-tiles per DMA batch
NKB = (KT + KB - 1) // KB

bf16 = ml_dtypes.bfloat16

_STAGES = {"e": 1, "x3": 2, "scan": 3, "scores": 4, "soft": 5, "ab": 6, "ctx": 6, "feat": 7, "full": 9}


def build_nc(upto="full"):
    """Emit + compile the Bass program (single NEFF, SPMD across 8 cores).
    upto < full truncates after that stage and dumps a debug view."""
    lvl = _STAGES[upto]
    nc = bacc.Bacc("TRN2", target_bir_lowering=False, debug=False,
                   enable_asserts=False)

    # ---- DRAM I/O ----
    d_xt = nc.dram_tensor("xt", [KT, 128, N], BF16, kind="ExternalInput").ap()
    d_emb = nc.dram_tensor("embhl", [KT, 128, 256], BF16, kind="ExternalInput").ap()
    d_wih_f = nc.dram_tensor("wihT_fhl", [2, 128, 384], BF16, kind="ExternalInput").ap()
    d_wih_b = nc.dram_tensor("wihT_bhl", [2, 128, 384], BF16, kind="ExternalInput").ap()
    d_whh_f = nc.dram_tensor("whhT_f16", [128, 384], BF16, kind="ExternalInput").ap()
    d_whh_b = nc.dram_tensor("whhT_b16", [128, 384], BF16, kind="ExternalInput").ap()
    d_brow = nc.dram_tensor("bias_rows_hl", [2, 1, 768], BF16, kind="ExternalInput").ap()
    d_bihn = nc.dram_tensor("bihn", [128, 2], F32, kind="ExternalInput").ap()
    d_attnw = nc.dram_tensor("attn_wc", [128, 2], F32, kind="ExternalInput").ap()
    d_attnb = nc.dram_tensor("attn_b4", [4, 1], F32, kind="ExternalInput").ap()
    d_combw = nc.dram_tensor("comb_wT", [128, 512], F32, kind="ExternalInput").ap()
    d_combb = nc.dram_tensor("comb_b", [128, 1], F32, kind="ExternalInput").ap()
    d_fcw = nc.dram_tensor("fc_wT", [128, OUT], F32, kind="ExternalInput").ap()
    d_fcb = nc.dram_tensor("fc_b", [1, OUT], F32, kind="ExternalInput").ap()
    d_iota = nc.dram_tensor("iota4", [4, T], F32, kind="ExternalInput").ap()
    d_out = nc.dram_tensor("logits", [BPC, OUT], F32, kind="ExternalOutput").ap()
    d_b1 = nc.dram_tensor("bounce1", [2 * N], F32, kind="Internal").ap()
    d_b2 = nc.dram_tensor("bounce2", [2 * N], F32, kind="Internal").ap()
    d_er = nc.dram_tensor("erev_scratch", [128, N], F32, kind="Internal").ap()

    from contextlib import ExitStack
    with tile.TileContext(nc) as tc, ExitStack() as ctx:
        cm_x3 = tc.tile_pool(name="p_x3", bufs=1, space="PSUM")
        p_x3 = cm_x3.__enter__()
        cm_e = tc.tile_pool(name="p_e", bufs=2, space="PSUM")
        p_e = cm_e.__enter__()
        sb_c = ctx.enter_context(tc.tile_pool(name="sb_c", bufs=1))
        sb_m = ctx.enter_context(tc.tile_pool(name="sb_m", bufs=1))
        sb_scan = ctx.enter_context(tc.tile_pool(name="sb_scan", bufs=2))
        sb_x = ctx.enter_context(tc.tile_pool(name="sb_x", bufs=4))
        sb_e = ctx.enter_context(tc.tile_pool(name="sb_e", bufs=4))

        # ---- constants into SBUF ----
        w_sb = sb_c.tile([128, 2, 2, 384], BF16)   # [dir][hi/lo] wih
        nc.sync.dma_start(out=w_sb[:, 0, :, :], in_=d_wih_f.rearrange("h p n -> p h n"))
        nc.sync.dma_start(out=w_sb[:, 1, :, :], in_=d_wih_b.rearrange("h p n -> p h n"))
        w16_sb = sb_c.tile([128, 2, 384], BF16)    # whhf, whhb (scan weights)
        nc.sync.dma_start(out=w16_sb[:, 0, :], in_=d_whh_f)
        nc.sync.dma_start(out=w16_sb[:, 1, :], in_=d_whh_b)
        brow_sb = sb_c.tile([1, 2, 768], BF16)     # [hi/lo] bias rows
        nc.sync.dma_start(out=brow_sb, in_=d_brow.rearrange("h p n -> p h n"))
        bihn_sb = sb_c.tile([128, 2], F32)
        nc.sync.dma_start(out=bihn_sb, in_=d_bihn)
        attnw_sb = sb_c.tile([128, 2], F32)
        nc.sync.dma_start(out=attnw_sb, in_=d_attnw)
        attnb_sb = sb_c.tile([4, 1], F32)
        nc.sync.dma_start(out=attnb_sb, in_=d_attnb)
        combw_sb = sb_c.tile([128, 512], F32)
        nc.sync.dma_start(out=combw_sb, in_=d_combw)
        combb_sb = sb_c.tile([128, 1], F32)
        nc.sync.dma_start(out=combb_sb, in_=d_combb)
        fcw_sb = sb_c.tile([128, OUT], F32)
        nc.sync.dma_start(out=fcw_sb, in_=d_fcw)
        fcb_sb = sb_c.tile([1, OUT], F32)
        nc.sync.dma_start(out=fcb_sb, in_=d_fcb)
        iota_sb = sb_c.tile([4, T], F32)
        nc.sync.dma_start(out=iota_sb, in_=d_iota)
        ones_sb = sb_c.tile([1, N], F32)
        nc.vector.memset(ones_sb, 1.0)
        ones16_sb = sb_c.tile([1, N], BF16)
        nc.vector.memset(ones16_sb, 1.0)
        onescol_sb = sb_c.tile([128, 1], F32)
        nc.vector.memset(onescol_sb, 1.0)

        # ---- long-lived SBUF state ----
        e_sb = sb_m.tile([128, N], F32)            # e.T, col = t*BPC + b
        xn_sb = sb_m.tile([128, 2, N], F32)        # xn + bih_n; dir b time-reversed
        HS = sb_m.tile([128, T + 1, 2, BPC], F32)  # [.,tau,0,.]=hf(tau-1), [.,tau,1,.]=hb(T-tau)
        HSb = sb_m.tile([128, T, BPC], F32)        # hb in true time order
        HC = sb_m.tile([128, T + 1, 2, BPC], BF16)  # bf16 mirror of HS for PE

        def dump(src_ap, nfree):
            dbg = sb_m.tile([BPC, OUT], F32)
            nc.vector.memset(dbg, 0.0)
            nc.vector.tensor_copy(dbg[:, 0:nfree], src_ap)
            nc.sync.dma_start(out=d_out, in_=dbg)

        # ---- phase 1: e.T accumulation in PSUM ----
        e_ps = p_e.tile([128, N], F32, tag="escratch")
        for kb in range(NKB):
            nk = min(KB, KT - kb * KB)
            xk = sb_x.tile([128, KB, N], BF16)
            nc.sync.dma_start(
                out=xk[:, :nk, :],
                in_=d_xt[kb * KB:kb * KB + nk].rearrange("k p n -> p k n"))
            ek = sb_e.tile([128, KB, 256], BF16)
            nc.sync.dma_start(
                out=ek[:, :nk, :],
                in_=d_emb[kb * KB:kb * KB + nk].rearrange("k p n -> p k n"))
            for j in range(nk):
                k = kb * KB + j
                nc.tensor.matmul(e_ps, ek[:, j, 0:128], xk[:, j, :],
                                 start=(k == 0), stop=False)
                nc.tensor.matmul(e_ps, ek[:, j, 128:256], xk[:, j, :],
                                 start=False, stop=(k == KT - 1))
        nc.scalar.copy(e_sb, e_ps)
        if lvl == 1:
            dump(e_sb[0:BPC, :], N)

        if lvl >= 2:
            # reversed-time copy of e.T via DRAM bounce (negative-stride
            # DRAM read APs are fine for DMA descriptors)
            e_rev = sb_m.tile([128, N], F32)
            nc.sync.dma_start(out=d_er, in_=e_sb)
            er_rev_ap = bass.AP(
                tensor=d_er.tensor, offset=d_er.offset + (T - 1) * BPC,
                ap=[list(d_er.ap[0]), [-BPC, T], [1, BPC]])
            nc.sync.dma_start(out=e_rev.rearrange("p (t b) -> p t b", b=BPC),
                              in_=er_rev_ap)
            # bf16 hi/lo casts of e and e_rev for the x3 matmuls
            ecast = sb_m.tile([128, 2, 2, N], BF16)   # [fwd/rev][hi/lo]
            for src_i, esrc in enumerate([e_sb, e_rev]):
                nc.vector.tensor_copy(ecast[:, src_i, 0, :], esrc)
                nc.vector.tensor_tensor(ecast[:, src_i, 1, :], esrc,
                                        ecast[:, src_i, 0, :], ALU.subtract)

            # ---- phase 2: x3 resident in PSUM + xn in SBUF ----
            # 3-term bf16 products: wh@eh + wh@el + wl@eh ~ fp32 exact
            rz_ps = p_x3.tile([128, 2, 2, 512], F32)   # [dir][gate r,z]
            n_ps = p_x3.tile([128, 2, 512], F32)       # [dir]

            def x3_mm(out_ap, di, g0, g1, first_start):
                terms = [(0, 0), (0, 1), (1, 0)]       # (w hi/lo, e hi/lo)
                for i, (wh, eh) in enumerate(terms):
                    nc.tensor.matmul(
                        out_ap, w_sb[:, di, wh, g0:g1], ecast[:, di, eh, :],
                        start=(first_start and i == 0), stop=True,
                        skip_group_check=not (first_start and i == 0))

            for di in range(2):
                for g in range(2):  # r, z
                    x3_mm(rz_ps[:, di, g, 0:N], di, g * 128, (g + 1) * 128,
                          True)
                    idx = di * 2 + g
                    for hl in range(2):
                        nc.tensor.matmul(
                            rz_ps[:, di, g, 0:N],
                            brow_sb[0:1, hl, idx * 128:(idx + 1) * 128],
                            ones16_sb,
                            start=False, stop=True, skip_group_check=True)
                for hl in range(2):
                    nc.tensor.matmul(
                        n_ps[:, di, 0:N],
                        brow_sb[0:1, hl, (4 + di) * 128:(5 + di) * 128],
                        ones16_sb, start=(hl == 0), stop=True,
                        skip_group_check=(hl != 0))
                xn_ps = p_e.tile([128, N], F32, tag="escratch")
                x3_mm(xn_ps, di, 256, 384, True)
                nc.scalar.add(xn_sb[:, di, :], xn_ps, bihn_sb[:, di:di + 1])
        cm_e.__exit__(None, None, None)
        if lvl == 2:
            dump(xn_sb[0:BPC, 0, :], N)

        if lvl >= 3:
            nc.vector.memset(HS[:, 0], 0.0)
            nc.vector.memset(HC[:, 0], 0.0)
            # ---- phase 3: GRU scan ----
            for t in range(T):
                hf = HC[:, t, 0, :]
                hb = HC[:, t, 1, :]
                c0, c1 = t * BPC, (t + 1) * BPC
                nc.tensor.matmul(rz_ps[:, 0, 0, c0:c1], w16_sb[:, 0, 0:128], hf,
                                 start=False, stop=True, skip_group_check=True)
                nc.tensor.matmul(rz_ps[:, 0, 1, c0:c1], w16_sb[:, 0, 128:256], hf,
                                 start=False, stop=True, skip_group_check=True)
                nc.tensor.matmul(rz_ps[:, 1, 0, c0:c1], w16_sb[:, 1, 0:128], hb,
                                 start=False, stop=True, skip_group_check=True)
                nc.tensor.matmul(rz_ps[:, 1, 1, c0:c1], w16_sb[:, 1, 128:256], hb,
                                 start=False, stop=True, skip_group_check=True)
                nc.tensor.matmul(n_ps[:, 0, c0:c1], w16_sb[:, 0, 256:384], hf,
                                 start=False, stop=True, skip_group_check=True)
                nc.tensor.matmul(n_ps[:, 1, c0:c1], w16_sb[:, 1, 256:384], hb,
                                 start=False, stop=True, skip_group_check=True)

                sig = sb_scan.tile([128, 2, 2, BPC], F32)
                nc.scalar.activation(sig, rz_ps[:, :, :, c0:c1], AF.Sigmoid)
                rn = sb_scan.tile([128, 2, BPC], F32)
                nc.vector.tensor_mul(rn, sig[:, :, 0, :], n_ps[:, :, c0:c1])
                arg = sb_scan.tile([128, 2, BPC], F32)
                nc.vector.tensor_add(arg, rn, xn_sb[:, :, c0:c1])
                # off-chain while tanh runs: zc = 1-z, w = z*h
                zc = sb_scan.tile([128, 2, BPC], F32)
                nc.vector.tensor_scalar(zc, sig[:, :, 1, :], -1.0, 1.0,
                                        ALU.mult, ALU.add)
                w = sb_scan.tile([128, 2, BPC], F32)
                nc.vector.tensor_mul(w, sig[:, :, 1, :], HS[:, t])
                nt = sb_scan.tile([128, 2, BPC], F32)
                nc.scalar.activation(nt, arg, AF.Tanh)
                m = sb_scan.tile([128, 2, BPC], F32)
                nc.vector.tensor_mul(m, zc, nt)
                nc.vector.tensor_add(HS[:, t + 1], m, w)
                nc.vector.tensor_add(HC[:, t + 1], m, w)
                nc.gpsimd.tensor_copy(HSb[:, T - 1 - t, :], HS[:, t + 1, 1, :])
        cm_x3.__exit__(None, None, None)
        if lvl == 3:
            dump(HSb[0:BPC, 0:50, :], 50 * BPC)

        if lvl >= 4:
            # ---- phase 4: attention + head ----
            p_a = ctx.enter_context(
                tc.tile_pool(name="p_a", bufs=1, space="PSUM"))
            hf_view = HS[:, 1:T + 1, 0, :]                 # [128, 100, 4]
            hb_view = HSb[:, :, :]

            abs_e = sb_m.tile([128, N], F32)
            nc.vector.tensor_mul(abs_e, e_sb, e_sb)
            sa_ps = p_a.tile([1, N], F32)
            nc.tensor.matmul(sa_ps, onescol_sb, abs_e, start=True, stop=True)
            s_ps = p_a.tile([1, N], F32)
            nc.tensor.matmul(s_ps, attnw_sb[:, 0:1], hf_view,
                             start=True, stop=False)
            nc.tensor.matmul(s_ps, attnw_sb[:, 1:2], hb_view,
                             start=False, stop=True)

            stage1 = sb_m.tile([1, 2, N], F32)
            nc.scalar.copy(stage1[:, 0, :], s_ps)
            nc.scalar.copy(stage1[:, 1, :], sa_ps)
            nc.sync.dma_start(out=d_b1, in_=stage1)
            s4 = sb_m.tile([4, 2, T], F32)
            # linear idx of (w, t, b) in bounce1 = w*400 + t*4 + b
            nc.sync.dma_start(
                out=s4, in_=d_b1.rearrange("(w t b) -> b w t", w=2, t=T, b=4))
            if lvl == 4:
                dump(s4[:, :, :], 2 * T)

        if lvl >= 5:
            pen4 = sb_m.tile([4, T], F32)
            nc.vector.tensor_scalar(pen4, s4[:, 1, :], 0.0, -1e9,
                                    ALU.is_equal, ALU.mult)
            m4 = sb_m.tile([4, T], F32)
            k4 = sb_m.tile([4, 1], F32)
            nc.vector.tensor_scalar(m4, s4[:, 1, :], 0.0, None, ALU.is_gt,
                                    op1=ALU.add, accum_out=k4)
            sm4 = sb_m.tile([4, T], F32)
            nc.vector.scalar_tensor_tensor(sm4, s4[:, 0, :], attnb_sb, pen4,
                                           ALU.add, ALU.add)
            negmax = sb_m.tile([4, 1], F32)
            nc.vector.reduce_max(negmax, sm4, AX.X, negate=True)
            easel = sb_m.tile([4, 2, T], F32)
            esum = sb_m.tile([4, 1], F32)
            nc.scalar.activation(easel[:, 0, :], sm4, AF.Exp, bias=negmax,
                                 accum_out=esum)
            rcp = sb_m.tile([4, 1], F32)
            nc.vector.reciprocal(rcp, esum)
            nc.vector.tensor_scalar_mul(easel[:, 0, :], easel[:, 0, :], rcp)
            nc.vector.tensor_scalar(easel[:, 1, :], iota_sb, k4, None,
                                    ALU.is_equal)
            nc.sync.dma_start(out=d_b2, in_=easel)
            flat2 = sb_m.tile([1, 2, T, 4], F32)
            # linear idx of (b, w, t) in bounce2 = b*200 + w*100 + t
            nc.sync.dma_start(
                out=flat2, in_=d_b2.rearrange("(b w t) -> w t b", b=4, w=2, t=T))
            if lvl == 5:
                dbg5 = sb_m.tile([BPC, OUT], F32)
                nc.vector.memset(dbg5, 0.0)
                nc.vector.tensor_copy(dbg5[0:1, 0:800], flat2[0:1, :, :, :])
                nc.sync.dma_start(out=d_out, in_=dbg5)

        if lvl >= 6:
            aB = p_a.tile([128, T, 4], F32)
            nc.tensor.matmul(aB, ones_sb[0:1, 0:128], flat2[:, 0],
                             start=True, stop=True)
            selB = p_a.tile([128, T, 4], F32)
            nc.tensor.matmul(selB, ones_sb[0:1, 0:128], flat2[:, 1],
                             start=True, stop=True)

            if upto == "ab":
                abd = sb_m.tile([BPC, OUT], F32)
                nc.vector.memset(abd, 0.0)
                nc.vector.tensor_copy(abd[:, 0:N], aB[0:BPC, :, :])
                nc.vector.tensor_copy(abd[:, N:2 * N], selB[0:BPC, :, :])
                nc.sync.dma_start(out=d_out, in_=abd)
            cc_sb = None if upto == "ab" else sb_m.tile([128, 4, BPC], F32)
            for blk, (wps, hview) in enumerate([] if upto == "ab" else
                    [(aB, hf_view), (aB, hb_view),
                     (selB, hf_view), (selB, hb_view)]):
                tmp = sb_scan.tile([128, T, BPC], F32, tag="ctx_tmp")
                nc.vector.tensor_mul(tmp, hview, wps)
                nc.vector.tensor_reduce(
                    cc_sb[:, blk, :], tmp.rearrange("p t b -> p b t"),
                    AX.X, ALU.add)

            if lvl == 6 and upto != "ab":
                dump(cc_sb[0:BPC, :, :], 16)

        if lvl >= 7:
            feat_ps = p_a.tile([128, BPC], F32)
            for i in range(4):
                nc.tensor.matmul(feat_ps, combw_sb[:, i * 128:(i + 1) * 128],
                                 cc_sb[:, i, :], start=(i == 0), stop=(i == 3))
            featT = sb_m.tile([128, BPC], F32)
            nc.scalar.activation(featT, feat_ps, AF.Tanh, bias=combb_sb)
            if lvl == 7:
                dump(featT[0:BPC, :], BPC)

        if lvl >= 8:
            lg0 = p_a.tile([BPC, 512], F32)
            nc.tensor.matmul(lg0, featT, fcw_sb[:, 0:512],
                             start=True, stop=False)
            nc.tensor.matmul(lg0, ones_sb[0:1, 0:BPC], fcb_sb[0:1, 0:512],
                             start=False, stop=True)
            lg1 = p_a.tile([BPC, OUT - 512], F32)
            nc.tensor.matmul(lg1, featT, fcw_sb[:, 512:OUT],
                             start=True, stop=False)
            nc.tensor.matmul(lg1, ones_sb[0:1, 0:BPC], fcb_sb[0:1, 512:OUT],
                             start=False, stop=True)
            out_sb = sb_m.tile([BPC, OUT], F32)
            nc.scalar.copy(out_sb[:, 0:512], lg0)
            nc.scalar.copy(out_sb[:, 512:OUT], lg1)
            nc.sync.dma_start(out=d_out, in_=out_sb)

    nc.compile()
    return nc


def prep_inputs(batchdata, emb, wih_f, whh_f, bih_f, bhh_f, wih_b, whh_b,
                bih_b, bhh_b, attn_w, attn_b, comb_w, comb_b, fc_w, fc_b):
    """Host-side sharding + layout prep. Returns per-core in_maps."""
    f32 = np.float32
    batchdata = np.asarray(batchdata, f32)
    emb = np.asarray(emb, f32)

    # emb hi/lo split, padded to VP rows, tiled [KT, 128, 256]
    hi = emb.astype(bf16)
    lo = (emb - hi.astype(f32)).astype(bf16)
    embhl = np.zeros((KT, 128, 256), bf16)
    hl = np.concatenate([hi, lo], axis=1)          # [V, 256]
    embhl.reshape(VP, 256)[:V] = hl

    def t_(a):
        return np.ascontiguousarray(np.asarray(a, f32).T)

    def hilo(a):
        h = a.astype(bf16)
        l = (a - h.astype(f32)).astype(bf16)
        return np.stack([h, l], axis=0)

    shared = {
        "embhl": embhl,
        "whhT_f16": t_(whh_f).astype(bf16), "whhT_b16": t_(whh_b).astype(bf16),
        "wihT_fhl": hilo(t_(wih_f)), "wihT_bhl": hilo(t_(wih_b)),
        "bias_rows_hl": hilo(np.concatenate([
            (np.asarray(bih_f, f32) + np.asarray(bhh_f, f32))[0:256],
            (np.asarray(bih_b, f32) + np.asarray(bhh_b, f32))[0:256],
            np.asarray(bhh_f, f32)[256:384],
            np.asarray(bhh_b, f32)[256:384],
        ]).reshape(1, 768)),
        "bihn": np.stack([np.asarray(bih_f, f32)[256:384],
                          np.asarray(bih_b, f32)[256:384]], axis=1).copy(),
        "attn_wc": np.ascontiguousarray(
            np.asarray(attn_w, f32).reshape(2, 128, 1)[:, :, 0].T),
        "attn_b4": np.full((4, 1), np.asarray(attn_b, f32).reshape(-1)[0], f32),
        "comb_wT": np.ascontiguousarray(
            t_(comb_w).reshape(4, 128, 128).transpose(1, 0, 2).reshape(128, 512)),
        "comb_b": np.asarray(comb_b, f32).reshape(128, 1).copy(),
        "fc_wT": t_(fc_w),
        "fc_b": np.asarray(fc_b, f32).reshape(1, OUT).copy(),
        "iota4": np.broadcast_to(
            np.arange(1, T + 1, dtype=f32)[None, :], (4, T)).copy(),
    }

    in_maps = []
    for c in range(NCORES):
        xc = batchdata[c * BPC:(c + 1) * BPC]       # [4, 100, V]
        # columns time-major: col j = t*BPC + b  ->  rows of [T, BPC, V]
        x2 = np.ascontiguousarray(
            xc.transpose(1, 0, 2).reshape(N, V).T.astype(bf16))  # [V, N]
        xt = np.zeros((KT, 128, N), bf16)
        xt.reshape(VP, N)[:V] = x2
        in_maps.append({"xt": xt, **shared})
    return in_maps


_NC_CACHE = {}


def get_compiled():
    if "nc" not in _NC_CACHE:
        nc = build_nc()
        nc.m = get_hw_module(nc.m)
        _NC_CACHE["nc"] = nc
    return _NC_CACHE["nc"]


def kernel(**inputs):
    nc = get_compiled()
    in_maps = prep_inputs(**inputs)
    res = bass_utils.run_bass_kernel_spmd(
        nc, in_maps, core_ids=list(range(NCORES)))
    out = np.concatenate([res.results[c]["logits"] for c in range(NCORES)],
                         axis=0)
    return out.astype(np.float32)
